# revision 2
# baseline (speedup 1.0000x reference)
"""AttnCutLoss on 8 TRN2 NeuronCores (pure data parallel over batch).

Same math as baseline: loss_b = -sum_j ln(out)*e^z / sum_j e^z with
z = (2/tau)*csum/(k+T) fetched as a host-precomputed reciprocal table row
(RTAB[T, j] = (2/tau)/(j+1+T), f16) via indirect row-gather keyed by T.

v2 changes (measured on HW):
  - ln runs as two quad-fused ACT instructions ([128, 8192] each, 0.85ns/elem)
    instead of eight per-tile ones (saves ~5us ACT).
  - optional CCE path (USE_CCE): the recip gather multiplies directly into the
    csum tile (DMA compute_op=mult), removing the z-multiply from DVE.
  - tighter engine assignment: DVE scan+TT only, ACT exp+s / ip accumulators.
"""

import os

import numpy as np

import bass_rust as _bass_rust
import concourse.bass as bass
import concourse.tile as tile
from concourse import bacc, mybir
from concourse.bass_utils import run_bass_kernel_spmd
from concourse.hw_specs import get_activation_tables

B, L = 8192, 2048
N_CORES = 8
ROWS_PER_CORE = B // N_CORES          # 1024
P = 128
TILES_PER_CORE = ROWS_PER_CORE // P   # 8
TAU = 0.95
VTAB = L + 1

USE_CCE = bool(int(os.environ.get("K2_USE_CCE", "0")))
N_STT = int(os.environ.get("K2_N_STT", "3"))
PAIR_SCAN = bool(int(os.environ.get("K2_PAIR_SCAN", "1")))

_CACHE = {}


def _pin_act_tables(nc):
    def patched(self):
        has_activation = any(
            isinstance(i, mybir.InstActivation)
            for b in self.main_func.blocks
            for i in b.instructions
        )
        if not has_activation:
            return
        AF = mybir.ActivationFunctionType
        keep = "natural_log_exp_and_others"
        tables = []
        for name, funcs in get_activation_tables(self.m.arch).items():
            if name != keep:
                funcs = {
                    f for f in funcs if f not in (AF.Exp, AF.Ln, AF.Copy)
                }
            tables.append((name, funcs))
        _bass_rust.insert_act_table_loads(self, tables)

    nc.insert_act_table_loads = patched.__get__(nc)


def _build_nc():
    f16 = mybir.dt.float16
    f32 = mybir.dt.float32
    i32 = mybir.dt.int32
    u8 = mybir.dt.uint8
    AF = mybir.ActivationFunctionType
    OP = mybir.AluOpType

    nc = bacc.Bacc("TRN2", target_bir_lowering=False, debug=False)
    _pin_act_tables(nc)
    labels_d = nc.dram_tensor(
        "labels", [ROWS_PER_CORE, L], f16 if PAIR_SCAN else u8, kind="ExternalInput"
    )
    outp_d = nc.dram_tensor("outp", [ROWS_PER_CORE, L], f16, kind="ExternalInput")
    rtab_d = nc.dram_tensor("rtab", [VTAB, L], f16, kind="ExternalInput")
    ip_d = nc.dram_tensor("ip_out", [P, TILES_PER_CORE], f32, kind="ExternalOutput")
    s_d = nc.dram_tensor("s_out", [P, TILES_PER_CORE], f32, kind="ExternalOutput")

    PAIRW = 2

    with tile.TileContext(nc) as tc:
        with (
            tc.tile_pool(name="lab", bufs=4) as labpool,
            tc.tile_pool(name="oq", bufs=3) as oqpool,
            tc.tile_pool(name="lo", bufs=3) as lopool,
            tc.tile_pool(name="work", bufs=4) as wpool,
            tc.tile_pool(name="e", bufs=3) as epool,
            tc.tile_pool(name="res", bufs=1) as rpool,
        ):
            ip_sb = rpool.tile([P, TILES_PER_CORE], f32)
            s_sb = rpool.tile([P, TILES_PER_CORE], f32)

            los = {}
            es = {}
            # software-pipelined: tile t's front half (scan/gather/z/exp) is
            # emitted at step t; its tail (w, ip) at step t+1 so neither DVE
            # nor ACT ever stalls its own stream waiting on the other engine.
            for t in range(TILES_PER_CORE + 1):
                if t < TILES_PER_CORE:
                    rows = slice(t * P, (t + 1) * P)
                    q = t % PAIRW

                    lab = labpool.tile([P, L], f16 if PAIR_SCAN else u8)
                    nc.sync.dma_start(lab[:], labels_d.ap()[rows, :])

                    if t % PAIRW == 0:
                        oq = oqpool.tile([P, PAIRW * L], f16)
                        for j in range(PAIRW):
                            rj = slice((t + j) * P, (t + j + 1) * P)
                            nc.sync.dma_start(
                                oq[:, j * L : (j + 1) * L], outp_d.ap()[rj, :]
                            )
                        lo = lopool.tile([P, PAIRW * L], f16)
                        nc.scalar.activation(lo[:], oq[:], AF.Ln)
                        los[t // PAIRW] = lo

                    csum = wpool.tile([P, L], f16)
                    if PAIR_SCAN:
                        # positions are host-permuted to [evens | odds].
                        # p = labE + labO; ps = cumsum(p) gives csum at odd
                        # positions; csum at evens = ps - labO.
                        H = L // 2
                        p = wpool.tile([P, H], f16)
                        nc.vector.tensor_tensor(
                            out=p[:], in0=lab[:, 0:H], in1=lab[:, H:L], op=OP.add
                        )
                        nc.vector.tensor_tensor_scan(
                            csum[:, H:L], p[:], p[:], 0.0, OP.add, OP.bypass
                        )
                        nc.vector.tensor_tensor(
                            out=csum[:, 0:H], in0=csum[:, H:L], in1=lab[:, H:L],
                            op=OP.subtract,
                        )
                    else:
                        nc.vector.tensor_tensor_scan(
                            csum[:], lab[:], lab[:], 0.0, OP.add, OP.bypass
                        )
                    offs = wpool.tile([P, 1], i32)
                    nc.vector.tensor_copy(offs[:], csum[:, L - 1 : L])

                    if USE_CCE:
                        nc.gpsimd.indirect_dma_start(
                            out=csum[:],
                            out_offset=None,
                            in_=rtab_d.ap(),
                            in_offset=bass.IndirectOffsetOnAxis(ap=offs[:, :1], axis=0),
                            compute_op=OP.mult,
                        )
                        z = csum
                    else:
                        rec = wpool.tile([P, L], f16)
                        nc.gpsimd.indirect_dma_start(
                            out=rec[:],
                            out_offset=None,
                            in_=rtab_d.ap(),
                            in_offset=bass.IndirectOffsetOnAxis(ap=offs[:, :1], axis=0),
                        )
                        z = wpool.tile([P, L], f16)
                        nc.vector.tensor_tensor(
                            out=z[:], in0=csum[:], in1=rec[:], op=OP.mult
                        )

                    e = epool.tile([P, L], f16)
                    nc.scalar.activation(e[:], z[:], AF.Exp, accum_out=s_sb[:, t : t + 1])
                    es[t] = e

                if t >= 1:
                    tp = t - 1
                    qp = tp % PAIRW
                    lop = los[tp // PAIRW]
                    ep = es.pop(tp)
                    if tp >= TILES_PER_CORE - N_STT:
                        # fused w = e*lo with ip accumulation, all on DVE
                        # (drains the tail without ACT ping-pong)
                        w = wpool.tile([P, L], f16)
                        nc.vector.scalar_tensor_tensor(
                            out=w[:], in0=ep[:], scalar=1.0,
                            in1=lop[:, qp * L : (qp + 1) * L],
                            op0=OP.mult, op1=OP.mult,
                            accum_out=ip_sb[:, tp : tp + 1],
                        )
                    else:
                        w = wpool.tile([P, L], f16)
                        nc.vector.tensor_tensor(
                            out=w[:], in0=ep[:], in1=lop[:, qp * L : (qp + 1) * L],
                            op=OP.mult,
                        )
                        wc = wpool.tile([P, L], f16)
                        nc.scalar.activation(
                            wc[:], w[:], AF.Copy, accum_out=ip_sb[:, tp : tp + 1]
                        )

            nc.sync.dma_start(ip_d.ap(), ip_sb[:])
            nc.sync.dma_start(s_d.ap(), s_sb[:])
    nc.compile()
    return nc


def _get_nc():
    key = ("nc", USE_CCE)
    if key not in _CACHE:
        _CACHE[key] = _build_nc()
    return _CACHE[key]


def _get_rtab():
    if "rtab" not in _CACHE:
        t = np.arange(VTAB, dtype=np.float64)[:, None]
        k = np.arange(1, L + 1, dtype=np.float64)[None, :]
        rtab = ((2.0 / TAU) / (k + t)).astype(np.float16)
        if PAIR_SCAN:
            rtab = np.concatenate([rtab[:, 0::2], rtab[:, 1::2]], axis=1)
        _CACHE["rtab"] = np.ascontiguousarray(rtab)
    return _CACHE["rtab"]


def _make_in_maps(output, labels):
    outp = np.asarray(output, dtype=np.float32).reshape(B, L).astype(np.float16)
    lab = np.asarray(labels).astype(np.uint8)
    if PAIR_SCAN:
        outp = np.concatenate([outp[:, 0::2], outp[:, 1::2]], axis=1)
        # ship [labE | labO] as f16 (u8 operands forgo the DVE 2x mode)
        lab = np.concatenate([lab[:, 0::2], lab[:, 1::2]], axis=1).astype(np.float16)
    rtab = _get_rtab()
    in_maps = []
    for c in range(N_CORES):
        rows = slice(c * ROWS_PER_CORE, (c + 1) * ROWS_PER_CORE)
        in_maps.append(
            {
                "labels": np.ascontiguousarray(lab[rows]),
                "outp": np.ascontiguousarray(outp[rows]),
                "rtab": rtab,
            }
        )
    return in_maps


def _reduce_results(results):
    total = 0.0
    for r in results:
        ip = r["ip_out"].astype(np.float64)
        s = r["s_out"].astype(np.float64)
        total += float((ip / s).sum())
    return np.float32(-total / B)


def kernel(output, labels):
    nc = _get_nc()
    in_maps = _make_in_maps(output, labels)
    res = run_bass_kernel_spmd(nc, in_maps, list(range(N_CORES)))
    return _reduce_results(res.results)


# revision 3
# speedup vs baseline: 1.0647x; 1.0647x over previous
"""AttnCutLoss on 8 TRN2 NeuronCores (pure data parallel over batch).

Same math as baseline: loss_b = -sum_j ln(out)*e^z / sum_j e^z with
z = (2/tau)*csum/(k+T) fetched as a host-precomputed reciprocal table row
(RTAB[T, j] = (2/tau)/(j+1+T), f16) via indirect row-gather keyed by T.

v2 changes (measured on HW):
  - ln runs as two quad-fused ACT instructions ([128, 8192] each, 0.85ns/elem)
    instead of eight per-tile ones (saves ~5us ACT).
  - optional CCE path (USE_CCE): the recip gather multiplies directly into the
    csum tile (DMA compute_op=mult), removing the z-multiply from DVE.
  - tighter engine assignment: DVE scan+TT only, ACT exp+s / ip accumulators.
"""

import numpy as np

import bass_rust as _bass_rust
import concourse.bass as bass
import concourse.tile as tile
from concourse import bacc, mybir
from concourse.bass_utils import run_bass_kernel_spmd
from concourse.hw_specs import get_activation_tables

B, L = 8192, 2048
N_CORES = 8
ROWS_PER_CORE = B // N_CORES          # 1024
P = 128
TILES_PER_CORE = ROWS_PER_CORE // P   # 8
TAU = 0.95
VTAB = L + 1

USE_CCE = False     # DMA CCE supports no mult with Copy mode (probed on HW)
N_STT = 3           # last 3 tiles: fused w*lo+accum on DVE drains the tail
PAIR_SCAN = True    # host even/odd split halves the DVE scan length

_CACHE = {}


def _pin_act_tables(nc):
    def patched(self):
        has_activation = any(
            isinstance(i, mybir.InstActivation)
            for b in self.main_func.blocks
            for i in b.instructions
        )
        if not has_activation:
            return
        AF = mybir.ActivationFunctionType
        keep = "natural_log_exp_and_others"
        tables = []
        for name, funcs in get_activation_tables(self.m.arch).items():
            if name != keep:
                funcs = {
                    f for f in funcs if f not in (AF.Exp, AF.Ln, AF.Copy)
                }
            tables.append((name, funcs))
        _bass_rust.insert_act_table_loads(self, tables)

    nc.insert_act_table_loads = patched.__get__(nc)


def _build_nc():
    f16 = mybir.dt.float16
    f32 = mybir.dt.float32
    i32 = mybir.dt.int32
    u8 = mybir.dt.uint8
    AF = mybir.ActivationFunctionType
    OP = mybir.AluOpType

    nc = bacc.Bacc("TRN2", target_bir_lowering=False, debug=False)
    _pin_act_tables(nc)
    labels_d = nc.dram_tensor(
        "labels", [ROWS_PER_CORE, L], f16 if PAIR_SCAN else u8, kind="ExternalInput"
    )
    outp_d = nc.dram_tensor("outp", [ROWS_PER_CORE, L], f16, kind="ExternalInput")
    rtab_d = nc.dram_tensor("rtab", [VTAB, L], f16, kind="ExternalInput")
    ip_d = nc.dram_tensor("ip_out", [P, TILES_PER_CORE], f32, kind="ExternalOutput")
    s_d = nc.dram_tensor("s_out", [P, TILES_PER_CORE], f32, kind="ExternalOutput")

    PAIRW = 2

    with tile.TileContext(nc) as tc:
        with (
            tc.tile_pool(name="lab", bufs=4) as labpool,
            tc.tile_pool(name="oq", bufs=3) as oqpool,
            tc.tile_pool(name="lo", bufs=3) as lopool,
            tc.tile_pool(name="work", bufs=4) as wpool,
            tc.tile_pool(name="e", bufs=3) as epool,
            tc.tile_pool(name="res", bufs=1) as rpool,
        ):
            ip_sb = rpool.tile([P, TILES_PER_CORE], f32)
            s_sb = rpool.tile([P, TILES_PER_CORE], f32)

            los = {}
            es = {}
            # software-pipelined: tile t's front half (scan/gather/z/exp) is
            # emitted at step t; its tail (w, ip) at step t+1 so neither DVE
            # nor ACT ever stalls its own stream waiting on the other engine.
            for t in range(TILES_PER_CORE + 1):
                if t < TILES_PER_CORE:
                    rows = slice(t * P, (t + 1) * P)
                    q = t % PAIRW

                    lab = labpool.tile([P, L], f16 if PAIR_SCAN else u8)
                    nc.sync.dma_start(lab[:], labels_d.ap()[rows, :])

                    if t % PAIRW == 0:
                        oq = oqpool.tile([P, PAIRW * L], f16)
                        for j in range(PAIRW):
                            rj = slice((t + j) * P, (t + j + 1) * P)
                            nc.sync.dma_start(
                                oq[:, j * L : (j + 1) * L], outp_d.ap()[rj, :]
                            )
                        lo = lopool.tile([P, PAIRW * L], f16)
                        nc.scalar.activation(lo[:], oq[:], AF.Ln)
                        los[t // PAIRW] = lo

                    csum = wpool.tile([P, L], f16)
                    if PAIR_SCAN:
                        # positions are host-permuted to [evens | odds].
                        # p = labE + labO; ps = cumsum(p) gives csum at odd
                        # positions; csum at evens = ps - labO.
                        H = L // 2
                        p = wpool.tile([P, H], f16)
                        nc.vector.tensor_tensor(
                            out=p[:], in0=lab[:, 0:H], in1=lab[:, H:L], op=OP.add
                        )
                        nc.vector.tensor_tensor_scan(
                            csum[:, H:L], p[:], p[:], 0.0, OP.add, OP.bypass
                        )
                        nc.vector.tensor_tensor(
                            out=csum[:, 0:H], in0=csum[:, H:L], in1=lab[:, H:L],
                            op=OP.subtract,
                        )
                    else:
                        nc.vector.tensor_tensor_scan(
                            csum[:], lab[:], lab[:], 0.0, OP.add, OP.bypass
                        )
                    offs = wpool.tile([P, 1], i32)
                    nc.vector.tensor_copy(offs[:], csum[:, L - 1 : L])

                    if USE_CCE:
                        nc.gpsimd.indirect_dma_start(
                            out=csum[:],
                            out_offset=None,
                            in_=rtab_d.ap(),
                            in_offset=bass.IndirectOffsetOnAxis(ap=offs[:, :1], axis=0),
                            compute_op=OP.mult,
                        )
                        z = csum
                    else:
                        rec = wpool.tile([P, L], f16)
                        nc.gpsimd.indirect_dma_start(
                            out=rec[:],
                            out_offset=None,
                            in_=rtab_d.ap(),
                            in_offset=bass.IndirectOffsetOnAxis(ap=offs[:, :1], axis=0),
                        )
                        z = wpool.tile([P, L], f16)
                        nc.vector.tensor_tensor(
                            out=z[:], in0=csum[:], in1=rec[:], op=OP.mult
                        )

                    e = epool.tile([P, L], f16)
                    nc.scalar.activation(e[:], z[:], AF.Exp, accum_out=s_sb[:, t : t + 1])
                    es[t] = e

                if t >= 1:
                    tp = t - 1
                    qp = tp % PAIRW
                    lop = los[tp // PAIRW]
                    ep = es.pop(tp)
                    if tp >= TILES_PER_CORE - N_STT:
                        # fused w = e*lo with ip accumulation, all on DVE
                        # (drains the tail without ACT ping-pong)
                        w = wpool.tile([P, L], f16)
                        nc.vector.scalar_tensor_tensor(
                            out=w[:], in0=ep[:], scalar=1.0,
                            in1=lop[:, qp * L : (qp + 1) * L],
                            op0=OP.mult, op1=OP.mult,
                            accum_out=ip_sb[:, tp : tp + 1],
                        )
                    else:
                        w = wpool.tile([P, L], f16)
                        nc.vector.tensor_tensor(
                            out=w[:], in0=ep[:], in1=lop[:, qp * L : (qp + 1) * L],
                            op=OP.mult,
                        )
                        wc = wpool.tile([P, L], f16)
                        nc.scalar.activation(
                            wc[:], w[:], AF.Copy, accum_out=ip_sb[:, tp : tp + 1]
                        )

            nc.sync.dma_start(ip_d.ap(), ip_sb[:])
            nc.sync.dma_start(s_d.ap(), s_sb[:])
    nc.compile()
    return nc


def _get_nc():
    key = ("nc", USE_CCE)
    if key not in _CACHE:
        _CACHE[key] = _build_nc()
    return _CACHE[key]


def _get_rtab():
    if "rtab" not in _CACHE:
        t = np.arange(VTAB, dtype=np.float64)[:, None]
        k = np.arange(1, L + 1, dtype=np.float64)[None, :]
        rtab = ((2.0 / TAU) / (k + t)).astype(np.float16)
        if PAIR_SCAN:
            rtab = np.concatenate([rtab[:, 0::2], rtab[:, 1::2]], axis=1)
        _CACHE["rtab"] = np.ascontiguousarray(rtab)
    return _CACHE["rtab"]


def _make_in_maps(output, labels):
    outp = np.asarray(output, dtype=np.float32).reshape(B, L).astype(np.float16)
    lab = np.asarray(labels).astype(np.uint8)
    if PAIR_SCAN:
        outp = np.concatenate([outp[:, 0::2], outp[:, 1::2]], axis=1)
        # ship [labE | labO] as f16 (u8 operands forgo the DVE 2x mode)
        lab = np.concatenate([lab[:, 0::2], lab[:, 1::2]], axis=1).astype(np.float16)
    rtab = _get_rtab()
    in_maps = []
    for c in range(N_CORES):
        rows = slice(c * ROWS_PER_CORE, (c + 1) * ROWS_PER_CORE)
        in_maps.append(
            {
                "labels": np.ascontiguousarray(lab[rows]),
                "outp": np.ascontiguousarray(outp[rows]),
                "rtab": rtab,
            }
        )
    return in_maps


def _reduce_results(results):
    total = 0.0
    for r in results:
        ip = r["ip_out"].astype(np.float64)
        s = r["s_out"].astype(np.float64)
        total += float((ip / s).sum())
    return np.float32(-total / B)


def kernel(output, labels):
    nc = _get_nc()
    in_maps = _make_in_maps(output, labels)
    res = run_bass_kernel_spmd(nc, in_maps, list(range(N_CORES)))
    return _reduce_results(res.results)


# revision 4
# speedup vs baseline: 1.0936x; 1.0272x over previous
"""AttnCutLoss on 8 TRN2 NeuronCores (pure data parallel over batch).

Same math as baseline: loss_b = -sum_j ln(out)*e^z / sum_j e^z with
z = (2/tau)*csum/(k+T) fetched as a host-precomputed reciprocal table row
(RTAB[T, j] = (2/tau)/(j+1+T), f16) via indirect row-gather keyed by T.

v2 changes (measured on HW):
  - ln runs as two quad-fused ACT instructions ([128, 8192] each, 0.85ns/elem)
    instead of eight per-tile ones (saves ~5us ACT).
  - optional CCE path (USE_CCE): the recip gather multiplies directly into the
    csum tile (DMA compute_op=mult), removing the z-multiply from DVE.
  - tighter engine assignment: DVE scan+TT only, ACT exp+s / ip accumulators.
"""

import numpy as np

import bass_rust as _bass_rust
import concourse.bass as bass
import concourse.tile as tile
from concourse import bacc, mybir
from concourse.bass_utils import run_bass_kernel_spmd
from concourse.hw_specs import get_activation_tables

B, L = 8192, 2048
N_CORES = 8
ROWS_PER_CORE = B // N_CORES          # 1024
P = 128
TILES_PER_CORE = ROWS_PER_CORE // P   # 8
TAU = 0.95
VTAB = L + 1

USE_CCE = False     # DMA CCE supports no mult with Copy mode (probed on HW)
N_STT = 3           # last 3 tiles: fused w*lo+accum on DVE drains the tail
PAIR_SCAN = True    # host even/odd split halves the DVE scan length
N_EARLY_T = 0       # ACT-accum early-T measured slower; disabled

_CACHE = {}


def _pin_act_tables(nc):
    def patched(self):
        has_activation = any(
            isinstance(i, mybir.InstActivation)
            for b in self.main_func.blocks
            for i in b.instructions
        )
        if not has_activation:
            return
        AF = mybir.ActivationFunctionType
        keep = "natural_log_exp_and_others"
        tables = []
        for name, funcs in get_activation_tables(self.m.arch).items():
            if name != keep:
                funcs = {
                    f for f in funcs if f not in (AF.Exp, AF.Ln, AF.Copy)
                }
            tables.append((name, funcs))
        _bass_rust.insert_act_table_loads(self, tables)

    nc.insert_act_table_loads = patched.__get__(nc)


def _build_nc():
    f16 = mybir.dt.float16
    f32 = mybir.dt.float32
    i32 = mybir.dt.int32
    u8 = mybir.dt.uint8
    AF = mybir.ActivationFunctionType
    OP = mybir.AluOpType

    nc = bacc.Bacc("TRN2", target_bir_lowering=False, debug=False)
    _pin_act_tables(nc)
    labels_d = nc.dram_tensor(
        "labels", [ROWS_PER_CORE, L], f16 if PAIR_SCAN else u8, kind="ExternalInput"
    )
    outp_d = nc.dram_tensor("outp", [ROWS_PER_CORE, L], f16, kind="ExternalInput")
    rtab_d = nc.dram_tensor("rtab", [VTAB, L], f16, kind="ExternalInput")
    ip_d = nc.dram_tensor("ip_out", [P, TILES_PER_CORE], f32, kind="ExternalOutput")
    s_d = nc.dram_tensor("s_out", [P, TILES_PER_CORE], f32, kind="ExternalOutput")

    PAIRW = 2

    with tile.TileContext(nc) as tc:
        with (
            tc.tile_pool(name="lab", bufs=4) as labpool,
            tc.tile_pool(name="oq", bufs=3) as oqpool,
            tc.tile_pool(name="lo", bufs=3) as lopool,
            tc.tile_pool(name="work", bufs=4) as wpool,
            tc.tile_pool(name="e", bufs=3) as epool,
            tc.tile_pool(name="res", bufs=1) as rpool,
        ):
            ip_sb = rpool.tile([P, TILES_PER_CORE], f32)
            s_sb = rpool.tile([P, TILES_PER_CORE], f32)

            los = {}
            es = {}
            # software-pipelined: tile t's front half (scan/gather/z/exp) is
            # emitted at step t; its tail (w, ip) at step t+1 so neither DVE
            # nor ACT ever stalls its own stream waiting on the other engine.
            for t in range(TILES_PER_CORE + 1):
                if t < TILES_PER_CORE:
                    rows = slice(t * P, (t + 1) * P)
                    q = t % PAIRW

                    lab = labpool.tile([P, L], f16 if PAIR_SCAN else u8)
                    nc.sync.dma_start(lab[:], labels_d.ap()[rows, :])

                    if t % PAIRW == 0:
                        oq = oqpool.tile([P, PAIRW * L], f16)
                        for j in range(PAIRW):
                            rj = slice((t + j) * P, (t + j + 1) * P)
                            nc.gpsimd.dma_start(
                                oq[:, j * L : (j + 1) * L], outp_d.ap()[rj, :]
                            )
                        lo = lopool.tile([P, PAIRW * L], f16)
                        nc.scalar.activation(lo[:], oq[:], AF.Ln)
                        los[t // PAIRW] = lo

                    csum = wpool.tile([P, L], f16)
                    early_T = PAIR_SCAN and t >= TILES_PER_CORE - N_EARLY_T
                    if PAIR_SCAN:
                        # positions are host-permuted to [evens | odds].
                        # p = labE + labO; ps = cumsum(p) gives csum at odd
                        # positions; csum at evens = ps - labO.
                        H = L // 2
                        p = wpool.tile([P, H], f16)
                        nc.vector.tensor_tensor(
                            out=p[:], in0=lab[:, 0:H], in1=lab[:, H:L], op=OP.add
                        )
                        offs = wpool.tile([P, 1], i32)
                        if early_T:
                            # drain-critical tiles: T = sum(p) on ACT while the
                            # scan runs, so the gather overlaps the scan
                            Tacc = wpool.tile([P, 1], f32)
                            pc = wpool.tile([P, H], f16)
                            nc.scalar.activation(
                                pc[:], p[:], AF.Copy, accum_out=Tacc[:]
                            )
                            nc.vector.tensor_copy(offs[:], Tacc[:])
                        nc.vector.tensor_tensor_scan(
                            csum[:, H:L], p[:], p[:], 0.0, OP.add, OP.bypass
                        )
                        nc.vector.tensor_tensor(
                            out=csum[:, 0:H], in0=csum[:, H:L], in1=lab[:, H:L],
                            op=OP.subtract,
                        )
                    else:
                        nc.vector.tensor_tensor_scan(
                            csum[:], lab[:], lab[:], 0.0, OP.add, OP.bypass
                        )
                    if not (PAIR_SCAN and early_T):
                        offs = wpool.tile([P, 1], i32)
                        nc.vector.tensor_copy(offs[:], csum[:, L - 1 : L])

                    if USE_CCE:
                        nc.gpsimd.indirect_dma_start(
                            out=csum[:],
                            out_offset=None,
                            in_=rtab_d.ap(),
                            in_offset=bass.IndirectOffsetOnAxis(ap=offs[:, :1], axis=0),
                            compute_op=OP.mult,
                        )
                        z = csum
                    else:
                        rec = wpool.tile([P, L], f16)
                        nc.gpsimd.indirect_dma_start(
                            out=rec[:],
                            out_offset=None,
                            in_=rtab_d.ap(),
                            in_offset=bass.IndirectOffsetOnAxis(ap=offs[:, :1], axis=0),
                        )
                        z = wpool.tile([P, L], f16)
                        nc.vector.tensor_tensor(
                            out=z[:], in0=csum[:], in1=rec[:], op=OP.mult
                        )

                    e = epool.tile([P, L], f16)
                    nc.scalar.activation(e[:], z[:], AF.Exp, accum_out=s_sb[:, t : t + 1])
                    es[t] = e

                if t >= 1:
                    tp = t - 1
                    qp = tp % PAIRW
                    lop = los[tp // PAIRW]
                    ep = es.pop(tp)
                    if tp >= TILES_PER_CORE - N_STT:
                        # fused w = e*lo with ip accumulation, all on DVE
                        # (drains the tail without ACT ping-pong)
                        w = wpool.tile([P, L], f16)
                        nc.vector.scalar_tensor_tensor(
                            out=w[:], in0=ep[:], scalar=1.0,
                            in1=lop[:, qp * L : (qp + 1) * L],
                            op0=OP.mult, op1=OP.mult,
                            accum_out=ip_sb[:, tp : tp + 1],
                        )
                    else:
                        w = wpool.tile([P, L], f16)
                        nc.vector.tensor_tensor(
                            out=w[:], in0=ep[:], in1=lop[:, qp * L : (qp + 1) * L],
                            op=OP.mult,
                        )
                        wc = wpool.tile([P, L], f16)
                        nc.scalar.activation(
                            wc[:], w[:], AF.Copy, accum_out=ip_sb[:, tp : tp + 1]
                        )

            nc.sync.dma_start(ip_d.ap(), ip_sb[:])
            nc.sync.dma_start(s_d.ap(), s_sb[:])
    nc.compile()
    return nc


def _get_nc():
    key = ("nc", USE_CCE)
    if key not in _CACHE:
        _CACHE[key] = _build_nc()
    return _CACHE[key]


def _get_rtab():
    if "rtab" not in _CACHE:
        t = np.arange(VTAB, dtype=np.float64)[:, None]
        k = np.arange(1, L + 1, dtype=np.float64)[None, :]
        rtab = ((2.0 / TAU) / (k + t)).astype(np.float16)
        if PAIR_SCAN:
            rtab = np.concatenate([rtab[:, 0::2], rtab[:, 1::2]], axis=1)
        _CACHE["rtab"] = np.ascontiguousarray(rtab)
    return _CACHE["rtab"]


def _make_in_maps(output, labels):
    outp = np.asarray(output, dtype=np.float32).reshape(B, L).astype(np.float16)
    lab = np.asarray(labels).astype(np.uint8)
    if PAIR_SCAN:
        outp = np.concatenate([outp[:, 0::2], outp[:, 1::2]], axis=1)
        # ship [labE | labO] as f16 (u8 operands forgo the DVE 2x mode)
        lab = np.concatenate([lab[:, 0::2], lab[:, 1::2]], axis=1).astype(np.float16)
    rtab = _get_rtab()
    in_maps = []
    for c in range(N_CORES):
        rows = slice(c * ROWS_PER_CORE, (c + 1) * ROWS_PER_CORE)
        in_maps.append(
            {
                "labels": np.ascontiguousarray(lab[rows]),
                "outp": np.ascontiguousarray(outp[rows]),
                "rtab": rtab,
            }
        )
    return in_maps


def _reduce_results(results):
    total = 0.0
    for r in results:
        ip = r["ip_out"].astype(np.float64)
        s = r["s_out"].astype(np.float64)
        total += float((ip / s).sum())
    return np.float32(-total / B)


def kernel(output, labels):
    nc = _get_nc()
    in_maps = _make_in_maps(output, labels)
    res = run_bass_kernel_spmd(nc, in_maps, list(range(N_CORES)))
    return _reduce_results(res.results)
